# revision 5
# baseline (speedup 1.0000x reference)
"""MultiLora batched-adapter matmul on 8 TRN2 NeuronCores.

Problem: x [8, 2048, 4096] f32, weight [16, 4096, 4096] f32,
adapter_ids [8] int64 -> y[b] = x[b] @ weight[adapter_ids[b]].T,
output [8, 2048, 4096] f32.

Sharding: data-parallel over the batch dim, one batch per core. The
adapter gather happens on the host (adapter_ids is host-visible), so
each core receives exactly one [4096, 4096] adapter matrix and one
[2048, 4096] activation slice; the device kernel is a single large
matmul per core.

Device kernel layout (per core): both operands are fed K-major
(pre-transposed on the host) so the contraction dim sits on SBUF
partitions with no on-device transpose. Matmuls run in float32r
(single-pass FP32-reduced, ~FP22 mantissa) which is 4x the throughput
of true fp32 on the PE array; accumulation stays fp32 in PSUM.

Loop structure: x is cached in SBUF one 1024-row block at a time
(split into 32 K-slabs so the next block's slabs can stream in as the
last n-tile of the previous block drains). For each 512-wide n-tile,
w streams through SBUF in 256KB K-slabs while 8 PSUM banks accumulate
the full K=4096 contraction K-contiguously (keeps the PE warm, no
HAM oscillation).
"""

import numpy as np

B, S, D_IN = 8, 2048, 4096
A, D_OUT = 16, 4096

P = 128
M_BLK = 1024            # x rows resident per outer block
N_TILE = 512            # PSUM free dim per matmul
KO = D_IN // P          # 32 K-subtiles
MB = S // M_BLK         # 2 outer blocks
MS = M_BLK // P         # 8 m-subtiles (PSUM banks) per block
NT = D_OUT // N_TILE    # 8 n-tiles

_CACHE = {}


def _build():
    if "nc" in _CACHE:
        return _CACHE["nc"]

    import concourse.mybir as mybir
    import concourse.tile as tile
    from concourse import bacc

    f32 = mybir.dt.float32
    f32r = mybir.dt.float32r

    nc = bacc.Bacc(None, target_bir_lowering=False, name="multilora")
    xt = nc.declare_dram_parameter("xt", [D_IN, S], f32r, isOutput=False)
    wt = nc.declare_dram_parameter("wt", [D_IN, D_OUT], f32r, isOutput=False)
    y = nc.declare_dram_parameter("y", [S, D_OUT], f32, isOutput=True)

    xtv = xt.rearrange("(ko ki) m -> ki ko m", ki=P)
    wtv = wt.rearrange("(ko ki) n -> ki ko n", ki=P)

    with tile.TileContext(nc) as tc:
        with (
            tc.tile_pool(name="xpool", bufs=KO + 14) as xpool,
            tc.tile_pool(name="wpool", bufs=6) as wpool,
            tc.tile_pool(name="opool", bufs=4) as opool,
            tc.tile_pool(name="psum", bufs=1, space="PSUM") as pp,
        ):
            for mb in range(MB):
                xs = []
                for ko in range(KO):
                    t = xpool.tile([P, M_BLK], f32r, tag="xs")
                    nc.sync.dma_start(
                        out=t[:], in_=xtv[:, ko, mb * M_BLK : (mb + 1) * M_BLK]
                    )
                    xs.append(t)
                for nt in range(NT):
                    psums = [
                        pp.tile([P, N_TILE], f32, name=f"ps{i}") for i in range(MS)
                    ]
                    for ko in range(KO):
                        wtile = wpool.tile([P, N_TILE], f32r, tag="w")
                        nc.scalar.dma_start(
                            out=wtile[:],
                            in_=wtv[:, ko, nt * N_TILE : (nt + 1) * N_TILE],
                        )
                        for ms in range(MS):
                            nc.tensor.matmul(
                                psums[ms][:],
                                lhsT=xs[ko][:, ms * P : (ms + 1) * P],
                                rhs=wtile[:],
                                start=(ko == 0),
                                stop=(ko == KO - 1),
                            )
                    for ms in range(MS):
                        ot = opool.tile([P, N_TILE], f32, tag="o")
                        nc.vector.tensor_copy(out=ot[:], in_=psums[ms][:])
                        nc.gpsimd.dma_start(
                            out=y[
                                mb * M_BLK + ms * P : mb * M_BLK + (ms + 1) * P,
                                nt * N_TILE : (nt + 1) * N_TILE,
                            ],
                            in_=ot[:],
                        )

    nc.compile()
    _CACHE["nc"] = nc
    return nc


def kernel(x, weight, adapter_ids, trace=False):
    from concourse.bass_utils import run_bass_kernel_spmd

    x = np.asarray(x)
    weight = np.asarray(weight)
    ids = np.asarray(adapter_ids).astype(np.int64)

    nc = _build()

    wt_cache = {}
    in_maps = []
    for b in range(B):
        aid = int(ids[b])
        if aid not in wt_cache:
            wt_cache[aid] = np.ascontiguousarray(
                weight[aid].T.astype(np.float32, copy=False)
            )
        xt_b = np.ascontiguousarray(x[b].T.astype(np.float32, copy=False))
        in_maps.append({"xt": xt_b, "wt": wt_cache[aid]})

    res = run_bass_kernel_spmd(nc, in_maps, core_ids=list(range(B)), trace=trace)
    out = np.stack([res.results[b]["y"] for b in range(B)], axis=0)
    if trace:
        _CACHE["last_exec_time_ns"] = res.exec_time_ns
        _CACHE["last_result"] = res
    return out


# revision 6
# speedup vs baseline: 1.0126x; 1.0126x over previous
"""MultiLora batched-adapter matmul on 8 TRN2 NeuronCores.

Problem: x [8, 2048, 4096] f32, weight [16, 4096, 4096] f32,
adapter_ids [8] int64 -> y[b] = x[b] @ weight[adapter_ids[b]].T,
output [8, 2048, 4096] f32.

Sharding: data-parallel over the batch dim, one batch per core. The
adapter gather happens on the host (adapter_ids is host-visible), so
each core receives exactly one [4096, 4096] adapter matrix and one
[2048, 4096] activation slice; the device kernel is a single large
matmul per core.

Device kernel layout (per core): both operands are fed K-major
(pre-transposed on the host) so the contraction dim sits on SBUF
partitions with no on-device transpose. Matmuls run in float32r
(single-pass FP32-reduced, ~FP22 mantissa) which is 4x the throughput
of true fp32 on the PE array; accumulation stays fp32 in PSUM.

Loop structure: x is cached in SBUF one 1024-row block at a time
(split into 32 K-slabs so the next block's slabs can stream in as the
last n-tile of the previous block drains). For each 512-wide n-tile,
w streams through SBUF in 256KB K-slabs while 8 PSUM banks accumulate
the full K=4096 contraction K-contiguously (keeps the PE warm, no
HAM oscillation).
"""

import numpy as np

B, S, D_IN = 8, 2048, 4096
A, D_OUT = 16, 4096

P = 128
M_BLK = 1024            # x rows resident per outer block
N_TILE = 512            # PSUM free dim per matmul
KO = D_IN // P          # 32 K-subtiles
MB = S // M_BLK         # 2 outer blocks
MS = M_BLK // P         # 8 m-subtiles (PSUM banks) per block
NT = D_OUT // N_TILE    # 8 n-tiles

_CACHE = {}


def _build():
    if "nc" in _CACHE:
        return _CACHE["nc"]

    import concourse.mybir as mybir
    import concourse.tile as tile
    from concourse import bacc

    f32 = mybir.dt.float32
    f32r = mybir.dt.float32r

    nc = bacc.Bacc(None, target_bir_lowering=False, name="multilora")
    xt = nc.declare_dram_parameter("xt", [D_IN, S], f32r, isOutput=False)
    wt = nc.declare_dram_parameter("wt", [D_IN, D_OUT], f32r, isOutput=False)
    y = nc.declare_dram_parameter("y", [S, D_OUT], f32, isOutput=True)

    xtv = xt.rearrange("(ko ki) m -> ki ko m", ki=P)
    wtv = wt.rearrange("(ko ki) n -> ki ko n", ki=P)

    with tile.TileContext(nc) as tc:
        with (
            tc.tile_pool(name="xpool", bufs=KO + 6) as xpool,
            tc.tile_pool(name="wpool", bufs=6) as wpool,
            tc.tile_pool(name="opool", bufs=6) as opool,
            tc.tile_pool(name="psum", bufs=1, space="PSUM") as pp,
        ):
            for mb in range(MB):
                xs = []
                for ko in range(KO):
                    t = xpool.tile([P, M_BLK], f32r, tag="xs")
                    nc.sync.dma_start(
                        out=t[:], in_=xtv[:, ko, mb * M_BLK : (mb + 1) * M_BLK]
                    )
                    xs.append(t)
                for nt in range(NT):
                    psums = [
                        pp.tile([P, N_TILE], f32, name=f"ps{i}") for i in range(MS)
                    ]
                    for ko in range(KO):
                        wtile = wpool.tile([P, N_TILE], f32r, tag="w")
                        nc.scalar.dma_start(
                            out=wtile[:],
                            in_=wtv[:, ko, nt * N_TILE : (nt + 1) * N_TILE],
                        )
                        for ms in range(MS):
                            nc.tensor.matmul(
                                psums[ms][:],
                                lhsT=xs[ko][:, ms * P : (ms + 1) * P],
                                rhs=wtile[:],
                                start=(ko == 0),
                                stop=(ko == KO - 1),
                            )
                    for ms in range(MS):
                        ot = opool.tile([P, N_TILE], f32, tag="o")
                        nc.vector.tensor_copy(out=ot[:], in_=psums[ms][:])
                        nc.gpsimd.dma_start(
                            out=y[
                                mb * M_BLK + ms * P : mb * M_BLK + (ms + 1) * P,
                                nt * N_TILE : (nt + 1) * N_TILE,
                            ],
                            in_=ot[:],
                        )

    nc.compile()
    _CACHE["nc"] = nc
    return nc


def kernel(x, weight, adapter_ids, trace=False):
    from concourse.bass_utils import run_bass_kernel_spmd

    x = np.asarray(x)
    weight = np.asarray(weight)
    ids = np.asarray(adapter_ids).astype(np.int64)

    nc = _build()

    wt_cache = {}
    in_maps = []
    for b in range(B):
        aid = int(ids[b])
        if aid not in wt_cache:
            wt_cache[aid] = np.ascontiguousarray(
                weight[aid].T.astype(np.float32, copy=False)
            )
        xt_b = np.ascontiguousarray(x[b].T.astype(np.float32, copy=False))
        in_maps.append({"xt": xt_b, "wt": wt_cache[aid]})

    res = run_bass_kernel_spmd(nc, in_maps, core_ids=list(range(B)), trace=trace)
    out = np.stack([res.results[b]["y"] for b in range(B)], axis=0)
    if trace:
        _CACHE["last_exec_time_ns"] = res.exec_time_ns
        _CACHE["last_result"] = res
    return out


# revision 7
# speedup vs baseline: 1.0458x; 1.0328x over previous
"""MultiLora batched-adapter matmul on 8 TRN2 NeuronCores.

Problem: x [8, 2048, 4096] f32, weight [16, 4096, 4096] f32,
adapter_ids [8] int64 -> y[b] = x[b] @ weight[adapter_ids[b]].T,
output [8, 2048, 4096] f32.

Sharding: data-parallel over the batch dim, one batch per core. The
adapter gather happens on the host (adapter_ids is host-visible), so
each core receives exactly one [4096, 4096] adapter matrix and one
[2048, 4096] activation slice; the device kernel is a single large
matmul per core.

Device kernel layout (per core): both operands are fed K-major
(pre-transposed on the host) so the contraction dim sits on SBUF
partitions with no on-device transpose. Matmuls run in float32r
(single-pass FP32-reduced, ~FP22 mantissa) which is 4x the throughput
of true fp32 on the PE array; accumulation stays fp32 in PSUM.

Loop structure: x is cached in SBUF one 1024-row block at a time
(split into 32 K-slabs so the next block's slabs can stream in as the
last n-tile of the previous block drains). For each 512-wide n-tile,
w streams through SBUF in 256KB K-slabs while 8 PSUM banks accumulate
the full K=4096 contraction K-contiguously (keeps the PE warm, no
HAM oscillation).
"""

import numpy as np

B, S, D_IN = 8, 2048, 4096
A, D_OUT = 16, 4096

P = 128
M_BLK = 1024            # x rows resident per outer block
N_TILE = 512            # PSUM free dim per matmul
KO = D_IN // P          # 32 K-subtiles
MB = S // M_BLK         # 2 outer blocks
MS = M_BLK // P         # 8 m-subtiles (PSUM banks) per block
NT = D_OUT // N_TILE    # 8 n-tiles

_CACHE = {}


def _build():
    if "nc" in _CACHE:
        return _CACHE["nc"]

    import concourse.mybir as mybir
    import concourse.tile as tile
    from concourse import bacc

    f32 = mybir.dt.float32
    f32r = mybir.dt.float32r

    nc = bacc.Bacc(None, target_bir_lowering=False, name="multilora")
    xt = nc.declare_dram_parameter("xt", [D_IN, S], f32r, isOutput=False)
    wt = nc.declare_dram_parameter("wt", [D_IN, D_OUT], f32r, isOutput=False)
    y = nc.declare_dram_parameter("y", [S, D_OUT], f32, isOutput=True)

    xtv = xt.rearrange("(ko ki) m -> ki ko m", ki=P)
    wtv = wt.rearrange("(ko ki) n -> ki ko n", ki=P)

    with tile.TileContext(nc) as tc:
        with (
            tc.tile_pool(name="xpool", bufs=KO + 6) as xpool,
            tc.tile_pool(name="wpool", bufs=6) as wpool,
            tc.tile_pool(name="opool", bufs=6) as opool,
            tc.tile_pool(name="psum", bufs=1, space="PSUM") as pp,
        ):
            for mb in range(MB):
                xs = []
                for ko in range(KO):
                    t = xpool.tile([P, M_BLK], f32r, tag="xs")
                    nc.sync.dma_start(
                        out=t[:], in_=xtv[:, ko, mb * M_BLK : (mb + 1) * M_BLK]
                    )
                    xs.append(t)
                for nt in range(NT):
                    psums = [
                        pp.tile([P, N_TILE], f32, name=f"ps{i}") for i in range(MS)
                    ]
                    for ko in range(KO):
                        wtile = wpool.tile([P, N_TILE], f32r, tag="w")
                        nc.scalar.dma_start(
                            out=wtile[:],
                            in_=wtv[:, ko, nt * N_TILE : (nt + 1) * N_TILE],
                        )
                        for ms in range(MS):
                            nc.tensor.matmul(
                                psums[ms][:],
                                lhsT=xs[ko][:, ms * P : (ms + 1) * P],
                                rhs=wtile[:],
                                start=(ko == 0),
                                stop=(ko == KO - 1),
                            )
                    for ms in range(MS):
                        ot = opool.tile([P, N_TILE], f32, tag="o")
                        nc.vector.tensor_copy(out=ot[:], in_=psums[ms][:])
                        nc.gpsimd.dma_start(
                            out=y[
                                mb * M_BLK + ms * P : mb * M_BLK + (ms + 1) * P,
                                nt * N_TILE : (nt + 1) * N_TILE,
                            ],
                            in_=ot[:],
                        )

    nc.compile()
    _CACHE["nc"] = nc
    return nc


def _run_in_maps(in_maps, trace=False):
    from concourse.bass_utils import run_bass_kernel_spmd

    nc = _build()
    res = run_bass_kernel_spmd(nc, in_maps, core_ids=list(range(B)), trace=trace)
    out = np.stack([res.results[b]["y"] for b in range(B)], axis=0)
    return out, res


def _retry_subprocess(in_maps):
    """Rerun the device execution in fresh subprocesses.

    The axon/NRT stack intermittently kills the first execute of a large
    NEFF with NRT_EXEC_UNIT_UNRECOVERABLE; once that happens the PJRT
    client in this process is poisoned, but a fresh process runs fine.
    """
    import os
    import subprocess
    import sys
    import tempfile

    for _attempt in range(3):
        tmpd = tempfile.mkdtemp(prefix="mlora_retry_")
        inp = os.path.join(tmpd, "in.npz")
        outp = os.path.join(tmpd, "out.npy")
        arrs = {}
        for b, m in enumerate(in_maps):
            arrs[f"xt{b}"] = m["xt"]
            arrs[f"wt{b}"] = m["wt"]
        np.savez(inp, **arrs)
        code = (
            "import numpy as np, importlib.util\n"
            f"spec = importlib.util.spec_from_file_location('mlora_kernel', {__file__!r})\n"
            "mod = importlib.util.module_from_spec(spec)\n"
            "spec.loader.exec_module(mod)\n"
            f"d = np.load({inp!r})\n"
            "in_maps = [{'xt': d[f'xt{b}'], 'wt': d[f'wt{b}']} for b in range(8)]\n"
            "out, _ = mod._run_in_maps(in_maps, False)\n"
            f"np.save({outp!r}, out)\n"
        )
        r = subprocess.run([sys.executable, "-c", code])
        if r.returncode == 0 and os.path.exists(outp):
            return np.load(outp)
    raise RuntimeError("kernel: device execution failed after 3 subprocess retries")


def kernel(x, weight, adapter_ids, trace=False):
    x = np.asarray(x)
    weight = np.asarray(weight)
    ids = np.asarray(adapter_ids).astype(np.int64)

    wt_cache = {}
    in_maps = []
    for b in range(B):
        aid = int(ids[b])
        if aid not in wt_cache:
            wt_cache[aid] = np.ascontiguousarray(
                weight[aid].T.astype(np.float32, copy=False)
            )
        xt_b = np.ascontiguousarray(x[b].T.astype(np.float32, copy=False))
        in_maps.append({"xt": xt_b, "wt": wt_cache[aid]})

    try:
        out, res = _run_in_maps(in_maps, trace=trace)
        if trace:
            _CACHE["last_exec_time_ns"] = res.exec_time_ns
            _CACHE["last_result"] = res
        return out
    except Exception:
        _CACHE["last_exec_time_ns"] = None
        return _retry_subprocess(in_maps)
